# revision 18
# baseline (speedup 1.0000x reference)
"""YOLOv3 head decode (DarkNet53.transform_grid_data) on 8 Trainium2 cores.

Input : features [32, 255, 76, 76] f32, anchor_size [6] f32
Output: [32, 17328, 85] f32, rows ordered (anchor, gy, gx), row layout
        [objness, box_x, box_y, box_w, box_h, conf*80].

Strategy: pure data-parallel over batch (4 batches/core). Per (batch,
anchor) plane the job is a [85, 5776] -> [5776, 85] f32 transpose with
pointwise transforms on 5 of 85 attr rows. Transpose runs on the PE
(identity matmul, SBUF->PSUM), DVE copies PSUM into an SBUF staging tile
whose partition p holds output rows [45p, 45p+45), so the store is one
fully contiguous DMA. The special attrs are fixed up post-transpose with
strided free-dim APs at full partition utilization:
  obj/x/y: sigmoid (ACT);  x/y: out = 8*sig + 8*grid  (fused DVE op)
  w/h:     8*anchor*exp(v) = exp(v + ln(8*anchor))    (ACT bias fold)

DMA queueing: loads and stores rotate across all three DMA queue
namespaces (nc.sync / nc.gpsimd / nc.scalar = SP-HWDGE / SWDGE /
ACT-HWDGE). Async-streamed HW marginals (back-to-back execs, noise-free
differential of 96- vs 192-plane variants) showed the 3-queue rotation
sustains ~231us per 12 planes vs ~253us when all loads are pinned to one
HWDGE ring and all stores to the other: real SDMA engines drain multiple
rings concurrently, and the DMA pattern itself (85-partition loads) runs
at ~205 GB/s effective, well under the 358 GB/s HBM model, so queue-level
parallelism matters more than completion ordering.
"""

import os
import sys

import numpy as np

try:
    import concourse.bass as bass
except ImportError:  # pragma: no cover
    sys.path.insert(0, "/opt/trn_rl_repo")
    import concourse.bass as bass

import concourse.bacc as bacc
import concourse.mybir as mybir
from concourse.bass_utils import run_bass_kernel_spmd
from concourse.tile import TileContext
from concourse.tile_rust import add_dep_helper

B = 32
A = 3
ATTR = 85
GH = GW = 76
NPIX = GH * GW            # 5776
STRIDE = 8                # 608 / 76
N_CORES = 8
B_LOC = B // N_CORES      # 4 batches per core
NPLANE = B_LOC * A        # 12 (batch, anchor) planes per core
K = 45                    # output rows per partition in the staging tile
NMAIN = 128 * K           # 5760 pixels via the main path
TAIL = NPIX - NMAIN       # 16 pixels via the tail path
QGRP = 5                  # transposes per PSUM bank (5*85*4B = 1700B < 2KB)

_f32 = mybir.dt.float32
_cache = {}


KA, KB = 22, 46           # balanced layout: 128*22 + 64*46 = 5760
NA = 128 * KA             # pixels via staging A (2816)


def _grid_xy8_bal():
    """8*gx, 8*gy tables for the balanced two-tier staging layout."""
    pix = np.arange(NPIX, dtype=np.int64)
    x8 = (STRIDE * (pix % GW)).astype(np.float32)
    y8 = (STRIDE * (pix // GW)).astype(np.float32)
    xy = np.stack([x8, y8], axis=-1)
    xa = np.ascontiguousarray(xy[:NA].reshape(128, KA * 2))
    xb = np.ascontiguousarray(xy[NA:NMAIN].reshape(64, KB * 2))
    return xa, xb


def _grid_xy8():
    """8*gx, 8*gy per pixel, in the staging layout [part, K, 2] + tail."""
    pix = np.arange(NPIX, dtype=np.int64)
    x8 = (STRIDE * (pix % GW)).astype(np.float32)
    y8 = (STRIDE * (pix // GW)).astype(np.float32)
    xy = np.stack([x8, y8], axis=-1)               # [5776, 2]
    main = xy[:NMAIN].reshape(128, K * 2)          # [128, 90]
    tail = xy[NMAIN:]                              # [16, 2]
    return np.ascontiguousarray(main), np.ascontiguousarray(tail)


def _build(nplane=NPLANE, do_specials=True, do_pe=True, do_load=True, do_store=True, pair_loads=False, io_bufs=5, st_bufs=4, alt_loads=3, alt_stores=3, balance=False, consts_gp=False, store_chunks=1):
    # Bacc (not plain Bass): TRN2 instructions carry at most ONE sync wait;
    # Bacc.generate_event_semaphores splits the extras into event-semaphore
    # instructions at finalize time.
    nc = bacc.Bacc("TRN2", target_bir_lowering=False, debug=False)
    feat = nc.dram_tensor("feat", [NPLANE, ATTR, NPIX], _f32, kind="ExternalInput")
    biaswh = nc.dram_tensor("biaswh", [128, 2 * A], _f32, kind="ExternalInput")
    outp = nc.dram_tensor("out", [NPLANE, NPIX, ATTR], _f32, kind="ExternalOutput")

    xy_main_np, xy_tail_np = _grid_xy8()
    xya_np, xyb_np = _grid_xy8_bal()
    ident_h = nc.inline_tensor(np.eye(ATTR, dtype=np.float32), name="ident")
    xym_h = nc.inline_tensor(xy_main_np, name="xym")
    xyt_h = nc.inline_tensor(xy_tail_np, name="xyt")
    xya_h = nc.inline_tensor(xya_np, name="xya")
    xyb_h = nc.inline_tensor(xyb_np, name="xyb")

    sig = mybir.ActivationFunctionType.Sigmoid
    exp = mybir.ActivationFunctionType.Exp
    mult = mybir.AluOpType.mult
    add = mybir.AluOpType.add

    with TileContext(nc) as tc:
        with (
            tc.tile_pool(name="consts", bufs=1) as cpool,
            tc.tile_pool(name="io", bufs=io_bufs) as iopool,
            tc.tile_pool(name="stg", bufs=st_bufs) as stpool,
            tc.tile_pool(name="ps", bufs=1, space="PSUM") as pspool,
            tc.tile_pool(name="pstail", bufs=1, space="PSUM") as ptpool,
            tc.tile_pool(name="pswarm", bufs=1, space="PSUM") as pwpool,
        ):
            ceng = nc.gpsimd if consts_gp else nc.sync
            id_t = cpool.tile([ATTR, ATTR], _f32)
            ceng.dma_start(out=id_t, in_=ident_h[:, :])
            bias_t = cpool.tile([128, 2 * A], _f32)
            ceng.dma_start(out=bias_t, in_=biaswh[:, :])
            xym_t = cpool.tile([128, K * 2], _f32)
            ceng.dma_start(out=xym_t, in_=xym_h[:, :])
            xyt_t = cpool.tile([TAIL, 2], _f32)
            ceng.dma_start(out=xyt_t, in_=xyt_h[:, :])
            xym3 = xym_t.rearrange("p (q c) -> p q c", c=2)
            if balance:
                xya_t = cpool.tile([128, KA * 2], _f32)
                ceng.dma_start(out=xya_t, in_=xya_h[:, :])
                xyb_t = cpool.tile([128, KB * 2], _f32)
                ceng.dma_start(out=xyb_t[64:128, :], in_=xyb_h[:, :])

            # fp32 self-loading matmuls (no standalone LDWEIGHTS) can carry
            # only ONE sync wait in the S3_LW struct; walrus rejects more.
            # Pool-recycled PSUM tiles impose release deps (PE completion of
            # old writers + DVE completion of the copy) = 2 waits on the
            # first matmul of each group. Instead allocate PSUM tiles ONCE
            # and rotate manually: the group-vs-group WAW is same-engine
            # (PE drains are pc-ordered -> safe, no wait emitted) and only
            # the WAR on the draining DVE copy remains. A per-plane
            # "absorber" transpose eats each input-DMA wait so plane-first
            # matmuls do not pair a DMA wait with the DVE wait. The PE
            # stream is pinned in emission order with ordering-only deps.
            pe_chain = [None]

            def pe_t(out_ap, in_ap, ident):
                inst = nc.tensor.transpose(out_ap, in_ap, ident)
                if pe_chain[0] is not None:
                    add_dep_helper(inst.ins, pe_chain[0].ins, sync=False,
                                   reason="pin PE order")
                pe_chain[0] = inst
                return inst

            warm = pwpool.tile([1, 2 * ATTR], _f32, tag="warm")
            pe_t(warm[:, :ATTR], id_t[:, 0:1], id_t)
            NPS = 3 if balance else 5
            psb_tiles = [
                pspool.tile([128, QGRP * ATTR], _f32, tag=f"psb{i}",
                            name=f"psb{i}")
                for i in range(3)
            ] if balance else []
            ps_tiles = [
                pspool.tile([128, QGRP * ATTR], _f32, tag=f"ps{i}", name=f"ps{i}")
                for i in range(NPS)
            ]
            pt_tiles = [
                ptpool.tile([TAIL, ATTR], _f32, tag=f"pt{i}", name=f"pt{i}")
                for i in range(1 if balance else 2)
            ]
            gctr = 0
            # all 12 plane tails accumulate here; one store at the end
            tails = stpool.tile(
                [TAIL, NPLANE * ATTR], _f32, tag="tails", bufs=1, name="tails"
            )

            pair_t = [None]
            for p in range(nplane):
                pm = p % NPLANE
                a = pm % A
                if pair_loads:
                    if p % 2 == 0:
                        pair = iopool.tile([ATTR, 2 * NPIX], _f32, tag="in2",
                                           name="pair")
                        if do_load:
                            nc.sync.dma_start(
                                out=pair.rearrange("k (j x) -> k j x", j=2),
                                in_=feat[pm : pm + 2].transpose([1, 0, 2]),
                            )
                            pe_t(warm[:, ATTR:], pair[:, 0:1], id_t)
                        pair_t[0] = pair
                    in_t = pair_t[0][
                        :, (p % 2) * NPIX : (p % 2 + 1) * NPIX
                    ]
                else:
                    in_t = iopool.tile([ATTR, NPIX], _f32, tag="in")
                    if do_load:
                        if alt_loads == 3:
                            eng = [nc.sync, nc.gpsimd, nc.scalar][p % 3]
                        else:
                            eng = nc.gpsimd if (alt_loads and p % 2) else nc.sync
                        eng.dma_start(out=in_t, in_=feat[pm])
                        pe_t(warm[:, ATTR:], in_t[:, 0:1], id_t)
                # [85, 128, K]: dim1 = staging partition, dim2 = row in part
                in_v = in_t[:, :NMAIN].rearrange("k (n q) -> k n q", q=K)

                if balance:
                    stA = stpool.tile([128, KA * ATTR], _f32, tag="stA")
                    stB = stpool.tile([128, KB * ATTR], _f32, tag="stB")
                    in_vA = in_t[:, :NA].rearrange("k (n q) -> k n q", q=KA)
                    in_vB = in_t[:, NA:NMAIN].rearrange(
                        "k (n q) -> k n q", q=KB
                    )
                    for part, qn, in_v2, st2, pstl in (
                        (slice(0, 128), KA, in_vA, stA, ps_tiles),
                        (slice(64, 128), KB, in_vB, stB, psb_tiles),
                    ):
                        ngrp = (qn + QGRP - 1) // QGRP
                        for g in range(ngrp):
                            qs = range(g * QGRP, min(qn, (g + 1) * QGRP))
                            ps_t = pstl[g % 3]
                            for i, q in enumerate(qs):
                                pe_t(
                                    ps_t[part, i * ATTR : (i + 1) * ATTR],
                                    in_v2[:, :, q], id_t,
                                )
                            nc.vector.tensor_copy(
                                st2[part,
                                    g * QGRP * ATTR : (g * QGRP + len(qs)) * ATTR],
                                ps_t[part, : len(qs) * ATTR],
                            )
                    if do_specials:
                        for part, qn, st2, xyt2 in (
                            (slice(0, 128), KA, stA, xya_t),
                            (slice(64, 128), KB, stB, xyb_t),
                        ):
                            s3 = st2.rearrange("n (q t) -> n q t", t=ATTR)[part]
                            sg = s3[:, :, 0:3]
                            nc.scalar.activation(sg, sg, exp, scale=-1.0)
                            nc.vector.tensor_scalar_add(sg, sg, 1.0)
                            nc.vector.reciprocal(sg, sg)
                            nc.scalar.activation(
                                s3[:, :, 3:4], s3[:, :, 3:4], exp,
                                bias=bias_t[part, 2 * a : 2 * a + 1],
                            )
                            nc.scalar.activation(
                                s3[:, :, 4:5], s3[:, :, 4:5], exp,
                                bias=bias_t[part, 2 * a + 1 : 2 * a + 2],
                            )
                            nc.vector.scalar_tensor_tensor(
                                s3[:, :, 1:3], s3[:, :, 1:3], 8.0,
                                xyt2.rearrange("p (q c) -> p q c", c=2)[part],
                                op0=mult, op1=add,
                            )
                    if do_store:
                        sengs = [nc.scalar, nc.sync, nc.gpsimd]
                        sengs[p % 3].dma_start(
                            out=outp[pm, :NA, :].rearrange(
                                "(n q) t -> n q t", q=KA),
                            in_=stA.rearrange("n (q t) -> n q t", t=ATTR),
                        )
                        sengs[(p + 1) % 3].dma_start(
                            out=outp[pm, NA:NMAIN, :].rearrange(
                                "(n q) t -> n q t", q=KB),
                            in_=stB.rearrange(
                                "n (q t) -> n q t", t=ATTR)[64:128],
                        )
                    # tail below shared with the standard path
                    pt_t = pt_tiles[p % len(pt_tiles)]
                    pe_t(pt_t, in_t[:, NMAIN:], id_t)
                    nc.vector.tensor_copy(
                        tails[:, pm * ATTR : (pm + 1) * ATTR], pt_t
                    )
                    continue
                st = stpool.tile([128, K * ATTR], _f32, tag="st")
                st3 = st.rearrange("n (q t) -> n q t", t=ATTR)
                if not do_pe:
                    # keep a load->store dep alive for ablations
                    nc.vector.tensor_copy(st[0:1, 0:ATTR], in_t[0:1, 0:ATTR])
                    if do_store:
                        seng = nc.scalar if alt_stores != 3 else [
                            nc.scalar, nc.sync, nc.gpsimd][p % 3]
                        seng.dma_start(
                            out=outp[pm, :NMAIN, :].rearrange(
                                "(n q) t -> n q t", q=K),
                            in_=st3,
                        )
                ngrp = K // QGRP
                gsplit = [
                    (ngrp * c // store_chunks, ngrp * (c + 1) // store_chunks)
                    for c in range(store_chunks)
                ]
                for g0, g1 in gsplit if do_pe else []:
                    for g in range(g0, g1):
                        ps_t = ps_tiles[gctr % NPS]
                        gctr += 1
                        for i in range(QGRP):
                            q = g * QGRP + i
                            pe_t(
                                ps_t[:, i * ATTR : (i + 1) * ATTR],
                                in_v[:, :, q], id_t,
                            )
                        nc.vector.tensor_copy(
                            st[:, g * QGRP * ATTR : (g + 1) * QGRP * ATTR], ps_t
                        )
                    q0, q1 = g0 * QGRP, g1 * QGRP
                    s3 = st3[:, q0:q1]
                    if do_specials:
                        # sigmoid(x) = 1/(1+exp(-x)) keeps only the Exp ACT
                        # table resident (sigmoid lives in a different table
                        # set; each switch costs ~1.3us of ACT time).
                        sg = s3[:, :, 0:3]
                        nc.scalar.activation(sg, sg, exp, scale=-1.0)
                        nc.vector.tensor_scalar_add(sg, sg, 1.0)
                        nc.vector.reciprocal(sg, sg)
                        nc.scalar.activation(
                            s3[:, :, 3:4], s3[:, :, 3:4], exp,
                            bias=bias_t[:, 2 * a : 2 * a + 1],
                        )
                        nc.scalar.activation(
                            s3[:, :, 4:5], s3[:, :, 4:5], exp,
                            bias=bias_t[:, 2 * a + 1 : 2 * a + 2],
                        )
                        nc.vector.scalar_tensor_tensor(
                            s3[:, :, 1:3], s3[:, :, 1:3], 8.0, xym3[:, q0:q1],
                            op0=mult, op1=add,
                        )
                    if do_store:
                        if alt_stores == 3:
                            seng = [nc.scalar, nc.sync, nc.gpsimd][p % 3]
                        else:
                            seng = nc.gpsimd if (alt_stores and p % 2) else nc.scalar
                        seng.dma_start(
                            out=outp[pm, :NMAIN, :].rearrange(
                                "(n q) t -> n q t", q=K)[:, q0:q1],
                            in_=s3,
                        )

                # 16-pixel tail: transpose into the batched tail tile
                if do_pe:
                    pt_t = pt_tiles[p % len(pt_tiles)]
                    pe_t(pt_t, in_t[:, NMAIN:], id_t)
                    nc.vector.tensor_copy(
                        tails[:, pm * ATTR : (pm + 1) * ATTR], pt_t
                    )

            # batched tail specials + one store for all 12 plane tails
            if do_pe and nplane >= NPLANE:
                tl3 = tails.rearrange("n (p t) -> n p t", t=ATTR)
                if do_specials:
                    sgt = tl3[:, :, 0:3]
                    nc.scalar.activation(sgt, sgt, exp, scale=-1.0)
                    nc.vector.tensor_scalar_add(sgt, sgt, 1.0)
                    nc.vector.reciprocal(sgt, sgt)
                    for a in range(A):
                        nc.scalar.activation(
                            tl3[:, a::A, 3:4], tl3[:, a::A, 3:4], exp,
                            bias=bias_t[:TAIL, 2 * a : 2 * a + 1],
                        )
                        nc.scalar.activation(
                            tl3[:, a::A, 4:5], tl3[:, a::A, 4:5], exp,
                            bias=bias_t[:TAIL, 2 * a + 1 : 2 * a + 2],
                        )
                    nc.vector.scalar_tensor_tensor(
                        tl3[:, :, 1:3], tl3[:, :, 1:3], 8.0,
                        xyt_t.unsqueeze(1).broadcast_to([TAIL, NPLANE, 2]),
                        op0=mult, op1=add,
                    )
                if do_store:
                    nc.scalar.dma_start(
                        out=outp[:, NMAIN:, :].transpose([1, 0, 2]), in_=tl3
                    )
    nc.finalize()
    return nc


def _get_nc(nplane=NPLANE, **kw):
    key = f"nc{nplane}{sorted(kw.items())}"
    if key not in _cache:
        _cache[key] = _build(nplane, **kw)
    return _cache[key]


def run(features, anchor_size, trace=False, **spmd_kwargs):
    features = np.ascontiguousarray(np.asarray(features, dtype=np.float32))
    anchor_size = np.asarray(anchor_size, dtype=np.float32)
    nc = _get_nc()

    # bias for the exp fold: w/h attr gets exp(v + ln(8*anchor))
    bias = np.log(8.0 * anchor_size.astype(np.float64)).astype(np.float32)
    biaswh = np.broadcast_to(bias, (128, 2 * A)).copy()

    in_maps = []
    for c in range(N_CORES):
        in_maps.append(
            {
                "feat": features[c * B_LOC : (c + 1) * B_LOC].reshape(
                    NPLANE, ATTR, NPIX
                ),
                "biaswh": biaswh,
            }
        )
    res = run_bass_kernel_spmd(
        nc, in_maps, list(range(N_CORES)), trace=trace, **spmd_kwargs
    )
    out = np.concatenate(
        [r["out"].reshape(B_LOC, A * NPIX, ATTR) for r in res.results], axis=0
    )
    return out, res


def kernel(features, anchor_size):
    out, _ = run(features, anchor_size)
    return out


def _prep_inputs(features, anchor_size):
    features = np.ascontiguousarray(np.asarray(features, dtype=np.float32))
    anchor_size = np.asarray(anchor_size, dtype=np.float32)
    bias = np.log(8.0 * anchor_size.astype(np.float64)).astype(np.float32)
    biaswh = np.broadcast_to(bias, (128, 2 * A)).copy()
    feats = [
        features[c * B_LOC : (c + 1) * B_LOC].reshape(NPLANE, ATTR, NPIX)
        for c in range(N_CORES)
    ]
    return feats, biaswh


def _make_exec(nplane=NPLANE, **kw):
    """Build a jitted single-exec callable over the 8-core mesh."""
    import jax
    from jax.sharding import Mesh, NamedSharding, PartitionSpec
    from jax.experimental.shard_map import shard_map

    from concourse import bass2jax as b2j

    nc = _get_nc(nplane, **kw)
    b2j.install_neuronx_cc_hook()
    part_name = nc.partition_id_tensor.name if nc.partition_id_tensor else None
    in_names, out_names, out_avals, zero_outs = [], [], [], []
    for alloc in nc.m.functions[0].allocations:
        if not isinstance(alloc, mybir.MemoryLocationSet):
            continue
        name = alloc.memorylocations[0].name
        if alloc.kind == "ExternalInput":
            if name != part_name:
                in_names.append(name)
        elif alloc.kind == "ExternalOutput":
            out_names.append(name)
            shape = tuple(alloc.tensor_shape)
            dtype = mybir.dt.np(alloc.dtype)
            out_avals.append(jax.core.ShapedArray(shape, dtype))
            zero_outs.append(np.zeros(shape, dtype))
    all_names = in_names + out_names + ([part_name] if part_name else [])

    def _body(*args):
        operands = list(args)
        if part_name:
            operands.append(b2j.partition_id_tensor())
        return tuple(
            b2j._bass_exec_p.bind(
                *operands,
                out_avals=tuple(out_avals),
                in_names=tuple(all_names),
                out_names=tuple(out_names),
                lowering_input_output_aliases=(),
                sim_require_finite=True,
                sim_require_nnan=True,
                nc=nc,
            )
        )

    devices = jax.devices()[:N_CORES]
    mesh = Mesh(np.asarray(devices), ("core",))
    nin = len(in_names) + len(zero_outs)
    f = jax.jit(
        shard_map(
            _body,
            mesh=mesh,
            in_specs=(PartitionSpec("core"),) * nin,
            out_specs=(PartitionSpec("core"),) * len(out_names),
            check_rep=False,
        ),
        keep_unused=True,
    )
    return f, in_names, zero_outs, mesh


def _make_chain_exec(k, nplane=NPLANE, **kw):
    """Jitted callable that runs the kernel NEFF k times back-to-back.

    Each exec's outputs feed the next exec's out-buffer operands, creating
    a strict data chain (no CSE, serial device execution). The marginal
    cost per added exec is the true kernel span + per-exec launch cost.
    """
    import jax
    from jax.sharding import Mesh, NamedSharding, PartitionSpec
    from jax.experimental.shard_map import shard_map

    from concourse import bass2jax as b2j

    nc = _get_nc(nplane, **kw)
    b2j.install_neuronx_cc_hook()
    part_name = nc.partition_id_tensor.name if nc.partition_id_tensor else None
    in_names, out_names, out_avals, zero_outs = [], [], [], []
    for alloc in nc.m.functions[0].allocations:
        if not isinstance(alloc, mybir.MemoryLocationSet):
            continue
        name = alloc.memorylocations[0].name
        if alloc.kind == "ExternalInput":
            if name != part_name:
                in_names.append(name)
        elif alloc.kind == "ExternalOutput":
            out_names.append(name)
            shape = tuple(alloc.tensor_shape)
            dtype = mybir.dt.np(alloc.dtype)
            out_avals.append(jax.core.ShapedArray(shape, dtype))
            zero_outs.append(np.zeros(shape, dtype))
    all_names = in_names + out_names + ([part_name] if part_name else [])

    def _chain(*args):
        ins = list(args[: len(in_names)])
        outs = list(args[len(in_names):])
        extra = [b2j.partition_id_tensor()] if part_name else []
        for _ in range(k):
            outs = list(
                b2j._bass_exec_p.bind(
                    *(ins + outs + extra),
                    out_avals=tuple(out_avals),
                    in_names=tuple(all_names),
                    out_names=tuple(out_names),
                    lowering_input_output_aliases=(),
                    sim_require_finite=True,
                    sim_require_nnan=True,
                    nc=nc,
                )
            )
        return tuple(outs)

    devices = jax.devices()[:N_CORES]
    mesh = Mesh(np.asarray(devices), ("core",))
    nin = len(in_names) + len(zero_outs)
    f = jax.jit(
        shard_map(
            _chain,
            mesh=mesh,
            in_specs=(PartitionSpec("core"),) * nin,
            out_specs=(PartitionSpec("core"),) * len(out_names),
            check_rep=False,
        ),
        keep_unused=True,
    )
    return f, in_names, zero_outs, mesh


def bench_chain(features, anchor_size, k=33, iters=24, reps=2, **kw):
    """Low-noise HW span estimate: (t_min(k execs) - t_min(1 exec)) / (k-1)."""
    import time

    import jax
    from jax.sharding import NamedSharding, PartitionSpec

    feats, biaswh = _prep_inputs(features, anchor_size)
    per_core = {"feat": feats, "biaswh": [biaswh] * N_CORES}

    runs = {}
    for kk in (1, k):
        f, in_names, zero_outs, mesh = _make_chain_exec(kk, **kw)
        concat_in = [np.concatenate(per_core[n], axis=0) for n in in_names]
        concat_zero = [
            np.zeros((N_CORES * z.shape[0], *z.shape[1:]), z.dtype)
            for z in zero_outs
        ]
        sh = NamedSharding(mesh, PartitionSpec("core"))
        dev_args = [jax.device_put(a, sh) for a in concat_in + concat_zero]
        jax.block_until_ready(dev_args)
        out = f(*dev_args)
        jax.block_until_ready(out)
        runs[kk] = (f, dev_args, out)

    tmins = {1: float("inf"), k: float("inf")}
    rep_diffs = []
    for _rep in range(reps):
        rm = {}
        for kk in (1, k):
            f, dev_args, _ = runs[kk]
            jax.block_until_ready(f(*dev_args))
            best = float("inf")
            for _ in range(iters):
                t0 = time.perf_counter()
                jax.block_until_ready(f(*dev_args))
                best = min(best, time.perf_counter() - t0)
            rm[kk] = best
            tmins[kk] = min(tmins[kk], best)
        rep_diffs.append((rm[k] - rm[1]) / (k - 1) * 1e9)
    bench_chain.rep_diffs = rep_diffs
    exec_ns = (tmins[k] - tmins[1]) / (k - 1) * 1e9
    out1 = runs[1][2]
    out = np.concatenate(
        [
            np.asarray(out1[0])
            .reshape(N_CORES, NPLANE, NPIX, ATTR)[c]
            .reshape(B_LOC, A * NPIX, ATTR)
            for c in range(N_CORES)
        ],
        axis=0,
    )
    return exec_ns, out, (tmins[1], tmins[k])


def bench(features, anchor_size, iters=6, blocks=12, big=8, **kw):
    """HW kernel span via a drift-robust interleaved differential.

    span(12 planes) = (t(big*12 planes) - t(12 planes)) / (big - 1), where
    the big variant runs the identical per-plane pipeline big times (plane
    index mod 12), so fixed host/dispatch overheads cancel exactly.

    The per-call wall-time floor drifts on the scale of seconds (axon link),
    so instead of two long phases we alternate short blocks of the two
    NEFFs, drop the first call after each model swap, take the min within
    each block, and report the median of per-block-pair differentials.
    """
    import time

    import jax
    from jax.sharding import NamedSharding, PartitionSpec

    feats, biaswh = _prep_inputs(features, anchor_size)
    per_core = {"feat": feats, "biaswh": [biaswh] * N_CORES}

    small, bigp = NPLANE, big * NPLANE
    runs = {}
    for npn in (small, bigp):
        f, in_names, zero_outs, mesh = _make_exec(npn, **kw)
        concat_in = [np.concatenate(per_core[n], axis=0) for n in in_names]
        concat_zero = [
            np.zeros((N_CORES * z.shape[0], *z.shape[1:]), z.dtype)
            for z in zero_outs
        ]
        sh = NamedSharding(mesh, PartitionSpec("core"))
        dev_args = [jax.device_put(a, sh) for a in concat_in + concat_zero]
        jax.block_until_ready(dev_args)
        out = f(*dev_args)
        jax.block_until_ready(out)
        runs[npn] = (f, dev_args, out)

    tmins = {small: float("inf"), bigp: float("inf")}
    diffs = []
    for _b in range(blocks):
        bm = {}
        for npn in (small, bigp):
            f, dev_args, _ = runs[npn]
            jax.block_until_ready(f(*dev_args))  # eat the model-swap cost
            best = float("inf")
            for _ in range(iters):
                t0 = time.perf_counter()
                jax.block_until_ready(f(*dev_args))
                best = min(best, time.perf_counter() - t0)
            bm[npn] = best
            tmins[npn] = min(tmins[npn], best)
        diffs.append((bm[bigp] - bm[small]) / (big - 1) * 1e9)
    bench.rep_diffs = diffs
    # Median of per-block differentials: robust to the per-call wall-time
    # noise (~+-1ms on an 80ms axon round trip) that makes min-based
    # differentials collapse to physically impossible values.
    diffs_s = sorted(diffs)
    exec_ns = diffs_s[len(diffs_s) // 2]

    out1 = runs[NPLANE][2]
    out = np.concatenate(
        [
            np.asarray(out1[0])
            .reshape(N_CORES, NPLANE, NPIX, ATTR)[c]
            .reshape(B_LOC, A * NPIX, ATTR)
            for c in range(N_CORES)
        ],
        axis=0,
    )
    return exec_ns, out, (tmins[small], tmins[bigp])

